# revision 5
# baseline (speedup 1.0000x reference)
"""Trainium2 Bass kernel for GQA attention (B=2, S=2048, H=2048, 32 Q heads,
8 KV heads, HD=64, RoPE, causal) with output projection.

Sharding: TP=4 over heads within each batch, DP=2 over batch -> 8 cores.
Core c handles batch c//4, head-rank c%4 (8 Q heads, 2 KV heads).
Each core computes a partial o_proj output [S, H]; the host sums the 4
partials per batch (cheaper than on-device all-reduce at these sizes).

Per-core layout (all transposed layouts produced by host as pure re-indexing):
  xt  [H, S]   = hidden[b].T                  fp32
  wqt [H, 512] = Wq[head-block r].T           fp32  (head order h0,h4,h1,h5,h2,h6,h3,h7)
  wkt [H, 128] = Wk[kv-block r].T             fp32
  wvt [H, 128] = Wv[kv-block r].T             fp32
  wot [512, H] = Wo[:, block r].T             fp32  (rows in same head order as wqt)
  c2/ss [128, S] RoPE cos/sin tables (two 64-row head blocks stacked)
  rot [128, 128] rotate-half permutation (+/-1) matrix, msk [128,128] causal bias

On device: cast to bf16, Q^T/K^T/V projections on PE, RoPE via PE rotation
matmul + DVE, scores^T = K^T.T Q^T per head (causal-trimmed), exp on ACT
(no max subtraction; scores are bounded ~|6|), AV with ones-augmented V to
get softmax denominators for free, normalize, o_proj. fp32 PSUM throughout.
"""

import numpy as np
from contextlib import ExitStack

import concourse.bass as bass
import concourse.bacc as bacc
import concourse.mybir as mybir
import concourse.tile as tile
from concourse.bass_utils import run_bass_kernel_spmd

F32 = mybir.dt.float32
BF16 = mybir.dt.bfloat16
AF = mybir.ActivationFunctionType

B, S, H = 2, 2048, 2048
NH, NKV, HD = 32, 8, 64
TP = 4                      # head-parallel ranks per batch
NQO = NH // TP * HD         # 512 per-core q features (8 heads)
NKO = NKV // TP * HD        # 128 per-core kv features (2 heads)
NHL = NH // TP              # 8 local q heads
EXP_SCALE = 1.0 / 8.0       # 1/sqrt(HD)
MASK_VAL = -30000.0
P = 128
QC = 512                    # q-chunk (one PSUM bank of fp32)
NSC = S // QC               # 4 q/s chunks
NPT = S // P                # 16 partition tiles of S
NHT = H // P                # 16 partition tiles of H

# local head h (0..7) -> (q-tile index, partition offset, local kv head)
# q heads are stored pairing (t, t+4) so that the d-partition offset of the
# q head always equals the d-partition offset of its kv head (PE array rows
# must line up between lhsT and rhs).
def _head_pos(h):
    g = h // 4               # local kv head, also partition block of K^T
    return h % 4, 64 * g, g


def build_nc():
    nc = bacc.Bacc("TRN2", target_bir_lowering=False, debug=False, num_devices=8)

    xt = nc.dram_tensor("xt", [H, S], F32, kind="ExternalInput").ap()
    wqt = nc.dram_tensor("wqt", [H, NQO], F32, kind="ExternalInput").ap()
    wkt = nc.dram_tensor("wkt", [H, NKO], F32, kind="ExternalInput").ap()
    wvt = nc.dram_tensor("wvt", [H, NKO], F32, kind="ExternalInput").ap()
    wot = nc.dram_tensor("wot", [NQO, H], F32, kind="ExternalInput").ap()
    c2 = nc.dram_tensor("c2", [P, S], F32, kind="ExternalInput").ap()
    ss = nc.dram_tensor("ss", [P, S], F32, kind="ExternalInput").ap()
    msk = nc.dram_tensor("msk", [P, P], F32, kind="ExternalInput").ap()
    rot = nc.dram_tensor("rot", [P, P], F32, kind="ExternalInput").ap()
    y = nc.dram_tensor("y", [S, H], F32, kind="ExternalOutput").ap()

    xt_t = xt.rearrange("(n p) s -> n p s", p=P)
    wqt_t = wqt.rearrange("(n p) o -> n p o", p=P)
    wkt_t = wkt.rearrange("(n p) o -> n p o", p=P)
    wvt_t = wvt.rearrange("(n p) o -> n p o", p=P)
    wot_t = wot.rearrange("(n p) o -> n p o", p=P)
    y_t = y.rearrange("(n p) o -> n p o", p=P)

    with tile.TileContext(nc) as tc, ExitStack() as ctx:
        persist = ctx.enter_context(tc.tile_pool(name="persist", bufs=1))

        c2_sb = persist.tile([P, S], F32, tag="c2", name="c2sb")
        ss_sb = persist.tile([P, S], F32, tag="ss", name="sssb")
        msk_sb = persist.tile([P, P], F32, tag="msk", name="msksb")
        rot_sb = persist.tile([P, P], F32, tag="rot", name="rotsb")
        ones65 = persist.tile([65, 64], F32, tag="ones65", name="ones65")
        nc.sync.dma_start(c2_sb[:], c2[:])
        nc.sync.dma_start(ss_sb[:], ss[:])
        nc.sync.dma_start(msk_sb[:], msk[:])
        nc.sync.dma_start(rot_sb[:], rot[:])
        nc.gpsimd.memset(ones65[64:65, :], 1.0)

        qtb = [persist.tile([P, S], BF16, tag=f"qtb{t}", name=f"qtb{t}") for t in range(4)]
        ktb = persist.tile([P, S], BF16, tag="ktb", name="ktb")
        vaug = [persist.tile([P, 130], BF16, tag=f"vaug{i}", name=f"vaug{i}") for i in range(NPT)]
        atb = [persist.tile([P, S], BF16, tag=f"atb{t}", name=f"atb{t}") for t in range(4)]
        wotb = [persist.tile([P, S], BF16, tag=f"wotb{t}", name=f"wotb{t}") for t in range(4)]
        wqtb = [persist.tile([P, NQO], BF16, tag=f"wqtb{i}", name=f"wqtb{i}") for i in range(NHT)]
        wktb = [persist.tile([P, NKO], BF16, tag=f"wktb{i}", name=f"wktb{i}") for i in range(NHT)]
        wvtb = [persist.tile([P, NKO], BF16, tag=f"wvtb{i}", name=f"wvtb{i}") for i in range(NHT)]

        # ------------- phase 1: load/cast inputs, QKV projections, RoPE ----
        with (
            tc.tile_pool(name="p1", bufs=2) as p1,
            tc.tile_pool(name="p1w", bufs=2) as p1w,
            tc.tile_pool(name="ps1", bufs=3, space="PSUM") as ps1,
            tc.tile_pool(name="ps1b", bufs=2, space="PSUM") as ps1b,
        ):
            xtb = [p1.tile([P, S], BF16, tag=f"xtb{i}", name=f"xtb{i}", bufs=1) for i in range(NHT)]
            for i in range(NHT):
                st = p1.tile([P, S], F32, tag="stage8k")
                nc.sync.dma_start(st[:], xt_t[i])
                nc.gpsimd.tensor_copy(xtb[i][:], st[:])
            for i in range(NHT):
                st = p1w.tile([P, NQO], F32, tag="wqstage")
                nc.sync.dma_start(st[:], wqt_t[i])
                nc.gpsimd.tensor_copy(wqtb[i][:], st[:])
                stk = p1w.tile([P, NKO], F32, tag="wkstage")
                nc.sync.dma_start(stk[:], wkt_t[i])
                nc.gpsimd.tensor_copy(wktb[i][:], stk[:])
                stv = p1w.tile([P, NKO], F32, tag="wvstage")
                nc.sync.dma_start(stv[:], wvt_t[i])
                nc.gpsimd.tensor_copy(wvtb[i][:], stv[:])
            for t in range(4):
                st = p1.tile([P, S], F32, tag="stage8k")
                nc.sync.dma_start(st[:], wot_t[t])
                nc.gpsimd.tensor_copy(wotb[t][:], st[:])

            def rope_tile(dst_ap, ps, sc):
                """RoPE: dst = ps*C2 + (R @ ps)*SS for one [128, 512] chunk."""
                ssl = slice(QC * sc, QC * (sc + 1))
                raw = p1.tile([P, QC], F32, tag="rope_raw")
                nc.scalar.copy(raw[:], ps[:])
                rps = ps1b.tile([P, QC], F32, tag="rps")
                nc.tensor.matmul(rps[:], lhsT=rot_sb[:], rhs=raw[:],
                                 start=True, stop=True)
                t1 = p1.tile([P, QC], F32, tag="rope_t1")
                nc.vector.tensor_mul(t1[:], raw[:], c2_sb[:, ssl])
                t2 = p1.tile([P, QC], F32, tag="rope_t2")
                nc.vector.tensor_mul(t2[:], rps[:], ss_sb[:, ssl])
                nc.vector.tensor_add(dst_ap, t1[:], t2[:])

            # Q^T: [512, S] as 4 tiles of [128(=2 heads), S]
            for t in range(4):
                for sc in range(NSC):
                    ps = ps1.tile([P, QC], F32, tag="mmps")
                    for i in range(NHT):
                        nc.tensor.matmul(
                            ps[:],
                            lhsT=wqtb[i][:, P * t:P * (t + 1)],
                            rhs=xtb[i][:, QC * sc:QC * (sc + 1)],
                            start=(i == 0), stop=(i == NHT - 1),
                        )
                    rope_tile(qtb[t][:, QC * sc:QC * (sc + 1)], ps, sc)
            # K^T: [128, S]
            for sc in range(NSC):
                ps = ps1.tile([P, QC], F32, tag="mmps")
                for i in range(NHT):
                    nc.tensor.matmul(
                        ps[:], lhsT=wktb[i][:], rhs=xtb[i][:, QC * sc:QC * (sc + 1)],
                        start=(i == 0), stop=(i == NHT - 1),
                    )
                rope_tile(ktb[:, QC * sc:QC * (sc + 1)], ps, sc)
            # V: [S, 128] as 16 tiles [128, 128]; augmented with ones cols
            for j in range(NPT):
                ps = ps1b.tile([P, NKO], F32, tag="vps")
                for i in range(NHT):
                    nc.tensor.matmul(
                        ps[:], lhsT=xtb[i][:, P * j:P * (j + 1)], rhs=wvtb[i][:],
                        start=(i == 0), stop=(i == NHT - 1),
                    )
                nc.scalar.copy(vaug[j][:, 0:64], ps[:, 0:64])
                nc.scalar.copy(vaug[j][:, 65:129], ps[:, 64:128])
                nc.gpsimd.memset(vaug[j][:, 64:65], 1.0)
                nc.gpsimd.memset(vaug[j][:, 129:130], 1.0)

        # ------------- phase 2: attention per local head ------------------
        with (
            tc.tile_pool(name="p2", bufs=6) as p2,
            tc.tile_pool(name="p2a", bufs=3) as p2a,
            tc.tile_pool(name="ps2s", bufs=3, space="PSUM") as ps2s,
            tc.tile_pool(name="ps2av", bufs=2, space="PSUM") as ps2av,
            tc.tile_pool(name="ps2b", bufs=2, space="PSUM") as ps2b,
        ):
            for h in range(NHL):
                t, off, g = _head_pos(h)
                for qc in range(NSC):
                    nkt = 4 * qc + 4       # k tiles this chunk sees
                    avp = ps2av.tile([65, QC], F32, tag="avp")
                    for ki in range(nkt):
                        j = ki - 4 * qc    # >=0 -> diagonal-band tile
                        col0 = P * j if j >= 0 else 0
                        sp = ps2s.tile([P, QC], F32, tag="sp")
                        nc.tensor.matmul(
                            sp[:, col0:QC],
                            lhsT=ktb[off:off + 64, P * ki:P * (ki + 1)],
                            rhs=qtb[t][off:off + 64, QC * qc + col0:QC * (qc + 1)],
                            start=True, stop=True,
                        )
                        if j >= 0:
                            nc.vector.tensor_add(
                                sp[:, col0:col0 + P], sp[:, col0:col0 + P], msk_sb[:]
                            )
                        ep = p2.tile([P, QC], BF16, tag="ep")
                        if col0 > 0:
                            nc.vector.memset(ep[:, 0:col0], 0.0)
                        nc.scalar.activation(
                            ep[:, col0:QC], sp[:, col0:QC], AF.Exp, scale=EXP_SCALE
                        )
                        nc.tensor.matmul(
                            avp[:],
                            lhsT=vaug[ki][:, 65 * g:65 * g + 65],
                            rhs=ep[:],
                            start=(ki == 0), stop=(ki == nkt - 1),
                        )
                    # normalize: A^T[h, qc] = avp[0:64] / avp[64]
                    atr = p2a.tile([65, QC], F32, tag="atr")
                    nc.scalar.copy(atr[:], avp[:])
                    nc.vector.reciprocal(atr[64:65, :], atr[64:65, :])
                    rbc = ps2b.tile([64, QC], F32, tag="rbc")
                    nc.tensor.matmul(
                        rbc[:], lhsT=ones65[64:65, :], rhs=atr[64:65, :],
                        start=True, stop=True,
                    )
                    nc.vector.tensor_mul(
                        atb[t][off:off + 64, QC * qc:QC * (qc + 1)],
                        atr[0:64, :], rbc[:],
                    )

        # ------------- phase 3: o_proj partial ----------------------------
        with (
            tc.tile_pool(name="p3", bufs=4) as p3,
            tc.tile_pool(name="ps3", bufs=4, space="PSUM") as ps3,
        ):
            for st in range(NPT):
                for oc in range(NSC):
                    op = ps3.tile([P, QC], F32, tag="op")
                    for ft in range(4):
                        nc.tensor.matmul(
                            op[:],
                            lhsT=atb[ft][:, P * st:P * (st + 1)],
                            rhs=wotb[ft][:, QC * oc:QC * (oc + 1)],
                            start=(ft == 0), stop=(ft == 3),
                        )
                    ost = p3.tile([P, QC], F32, tag="ost")
                    nc.scalar.copy(ost[:], op[:])
                    nc.sync.dma_start(y_t[st][:, QC * oc:QC * (oc + 1)], ost[:])

    nc.compile()
    return nc


def _host_tables():
    inv_freq = 1.0 / (10000.0 ** (np.arange(0, HD, 2, dtype=np.float32) / HD))
    pos = np.arange(S, dtype=np.float32)
    freqs = np.einsum('s,d->sd', pos, inv_freq)          # [S, 32]
    emb = np.concatenate([freqs, freqs], axis=-1)        # [S, 64]
    cosT = np.cos(emb).T.astype(np.float32)              # [64, S]
    sinT = np.sin(emb).T.astype(np.float32)
    c2 = np.ascontiguousarray(np.vstack([cosT, cosT]))   # [128, S]
    # sign of rotate_half is encoded in the rot matrix below; ss is plain sin
    ss = np.ascontiguousarray(np.vstack([sinT, sinT]))
    # rotate-half as a matmul: out[d] = sum_d' R[d', d] * in[d']
    R64 = np.zeros((HD, HD), dtype=np.float32)
    for d in range(32):
        R64[d + 32, d] = -1.0       # out[d] = -in[d+32]
        R64[d, d + 32] = 1.0        # out[d+32] = in[d]
    rot = np.zeros((P, P), dtype=np.float32)
    rot[0:64, 0:64] = R64
    rot[64:128, 64:128] = R64
    # causal bias for a diagonal 128x128 tile in scores^T[k, q] layout
    kk = np.arange(P)[:, None]
    qq = np.arange(P)[None, :]
    msk = np.where(kk <= qq, 0.0, MASK_VAL).astype(np.float32)
    return c2, ss, rot, msk


# q/o head order within a rank block: pair heads (u, u+4) in each 128-row tile
_HEAD_ORDER = [0, 4, 1, 5, 2, 6, 3, 7]


def _make_in_maps(hidden_states, Wq, Wk, Wv, Wo):
    hs = np.ascontiguousarray(np.asarray(hidden_states, dtype=np.float32))
    Wq = np.asarray(Wq, dtype=np.float32)
    Wk = np.asarray(Wk, dtype=np.float32)
    Wv = np.asarray(Wv, dtype=np.float32)
    Wo = np.asarray(Wo, dtype=np.float32)
    c2, ss, rot, msk = _host_tables()
    in_maps = []
    for c in range(8):
        b, r = c // 4, c % 4
        # row indices of Wq (= cols of Wo) for this rank, in device head order
        qrows = np.concatenate([
            np.arange(HD) + (NHL * r + u) * HD for u in _HEAD_ORDER
        ])
        in_maps.append({
            "xt": np.ascontiguousarray(hs[b].T),
            "wqt": np.ascontiguousarray(Wq[qrows, :].T),
            "wkt": np.ascontiguousarray(Wk[NKO * r:NKO * (r + 1), :].T),
            "wvt": np.ascontiguousarray(Wv[NKO * r:NKO * (r + 1), :].T),
            "wot": np.ascontiguousarray(Wo[:, qrows].T),
            "c2": c2, "ss": ss, "msk": msk, "rot": rot,
        })
    return in_maps


_NC = None


def _get_nc():
    global _NC
    if _NC is None:
        _NC = build_nc()
    return _NC


def run_cores(hidden_states, Wq, Wk, Wv, Wo, **run_kwargs):
    """Run the SPMD kernel; returns (out [B,S,H] fp32, BassKernelResults)."""
    nc = _get_nc()
    in_maps = _make_in_maps(hidden_states, Wq, Wk, Wv, Wo)
    res = run_bass_kernel_spmd(nc, in_maps, list(range(8)), **run_kwargs)
    out = np.zeros((B, S, H), dtype=np.float32)
    for c in range(8):
        out[c // 4] += res.results[c]["y"]
    return out, res


def kernel(hidden_states, Wq, Wk, Wv, Wo):
    out, _ = run_cores(hidden_states, Wq, Wk, Wv, Wo)
    return out


# revision 7
# speedup vs baseline: 1.3342x; 1.3342x over previous
"""Trainium2 Bass kernel for GQA attention (B=2, S=2048, H=2048, 32 Q heads,
8 KV heads, HD=64, RoPE, causal) with output projection.

Sharding: TP=4 over heads within each batch, DP=2 over batch -> 8 cores.
Core c handles batch c//4, head-rank c%4 (8 Q heads, 2 KV heads).
Each core computes a partial o_proj output [S, H]; the host sums the 4
partials per batch (cheaper than on-device all-reduce at these sizes).

Per-core layout (all transposed layouts produced by host as pure re-indexing):
  xt  [H, S]   = hidden[b].T                  fp32
  wqt [H, 512] = Wq[head-block r].T           fp32  (head order h0,h4,h1,h5,h2,h6,h3,h7)
  wkt [H, 128] = Wk[kv-block r].T             fp32
  wvt [H, 128] = Wv[kv-block r].T             fp32
  wot [512, H] = Wo[:, block r].T             fp32  (rows in same head order as wqt)
  c2/ss [128, S] RoPE cos/sin tables (two 64-row head blocks stacked)
  rot [128, 128] rotate-half permutation (+/-1) matrix, msk [128,128] causal bias

On device: cast to bf16, Q^T/K^T/V projections on PE, RoPE via PE rotation
matmul + DVE, scores^T = K^T.T Q^T per head (causal-trimmed), exp on ACT
(no max subtraction; scores are bounded ~|6|), AV with ones-augmented V to
get softmax denominators for free, normalize, o_proj. fp32 PSUM throughout.
"""

import numpy as np
from contextlib import ExitStack

import concourse.bass as bass
import concourse.bacc as bacc
import concourse.mybir as mybir
import concourse.tile as tile
from concourse.bass_utils import run_bass_kernel_spmd

F32 = mybir.dt.float32
BF16 = mybir.dt.bfloat16
AF = mybir.ActivationFunctionType

B, S, H = 2, 2048, 2048
NH, NKV, HD = 32, 8, 64
TP = 4                      # head-parallel ranks per batch
NQO = NH // TP * HD         # 512 per-core q features (8 heads)
NKO = NKV // TP * HD        # 128 per-core kv features (2 heads)
NHL = NH // TP              # 8 local q heads
EXP_SCALE = 1.0 / 8.0       # 1/sqrt(HD)
MASK_VAL = -30000.0
P = 128
QC = 512                    # q-chunk (one PSUM bank of fp32)
NSC = S // QC               # 4 q/s chunks
NPT = S // P                # 16 partition tiles of S
NHT = H // P                # 16 partition tiles of H

# local head h (0..7) -> (q-tile index, partition offset, local kv head)
# q heads are stored pairing (t, t+4) so that the d-partition offset of the
# q head always equals the d-partition offset of its kv head (PE array rows
# must line up between lhsT and rhs).
def _head_pos(h):
    g = h // 4               # local kv head, also partition block of K^T
    return h % 4, 64 * g, g


def build_nc():
    nc = bacc.Bacc("TRN2", target_bir_lowering=False, debug=False, num_devices=8)

    xt = nc.dram_tensor("xt", [H, S], F32, kind="ExternalInput").ap()
    wqt = nc.dram_tensor("wqt", [H, NQO], F32, kind="ExternalInput").ap()
    wkt = nc.dram_tensor("wkt", [H, NKO], F32, kind="ExternalInput").ap()
    wvt = nc.dram_tensor("wvt", [H, NKO], F32, kind="ExternalInput").ap()
    wot = nc.dram_tensor("wot", [NQO, H], F32, kind="ExternalInput").ap()
    c2 = nc.dram_tensor("c2", [P, S], F32, kind="ExternalInput").ap()
    ss = nc.dram_tensor("ss", [P, S], F32, kind="ExternalInput").ap()
    msk = nc.dram_tensor("msk", [P, P], F32, kind="ExternalInput").ap()
    rot = nc.dram_tensor("rot", [P, P], BF16, kind="ExternalInput").ap()
    y = nc.dram_tensor("y", [S, H], F32, kind="ExternalOutput").ap()

    xt_t = xt.rearrange("(n p) s -> n p s", p=P)
    wqt_t = wqt.rearrange("(n p) o -> n p o", p=P)
    wkt_t = wkt.rearrange("(n p) o -> n p o", p=P)
    wvt_t = wvt.rearrange("(n p) o -> n p o", p=P)
    wot_t = wot.rearrange("(n p) o -> n p o", p=P)
    y_t = y.rearrange("(n p) o -> n p o", p=P)

    with tile.TileContext(nc) as tc, ExitStack() as ctx:
        persist = ctx.enter_context(tc.tile_pool(name="persist", bufs=1))

        c2_sb = persist.tile([P, S], F32, tag="c2", name="c2sb")
        ss_sb = persist.tile([P, S], F32, tag="ss", name="sssb")
        msk_sb = persist.tile([P, P], F32, tag="msk", name="msksb")
        rot_sb = persist.tile([P, P], BF16, tag="rot", name="rotsb")
        ones65 = persist.tile([65, 64], F32, tag="ones65", name="ones65")
        nc.sync.dma_start(c2_sb[:], c2[:])
        nc.sync.dma_start(ss_sb[:], ss[:])
        nc.sync.dma_start(msk_sb[:], msk[:])
        nc.sync.dma_start(rot_sb[:], rot[:])
        nc.gpsimd.memset(ones65[64:65, :], 1.0)

        # chunked persistent activations: [tile][chunk] of [128, 512]
        qtbc = [[persist.tile([P, QC], BF16, tag=f"qtbc{t}_{sc}", name=f"qtbc{t}_{sc}")
                 for sc in range(NSC)] for t in range(4)]
        ktbc = [persist.tile([P, QC], BF16, tag=f"ktbc{sc}", name=f"ktbc{sc}")
                for sc in range(NSC)]
        vaug = [persist.tile([P, 130], BF16, tag=f"vaug{i}", name=f"vaug{i}")
                for i in range(NPT)]
        atbc = [[persist.tile([P, QC], BF16, tag=f"atbc{t}_{qc}", name=f"atbc{t}_{qc}")
                 for qc in range(NSC)] for t in range(4)]
        wotb = [persist.tile([P, S], BF16, tag=f"wotb{t}", name=f"wotb{t}") for t in range(4)]
        wqtb = [persist.tile([P, NQO], BF16, tag=f"wqtb{i}", name=f"wqtb{i}") for i in range(NHT)]
        wktb = [persist.tile([P, NKO], BF16, tag=f"wktb{i}", name=f"wktb{i}") for i in range(NHT)]
        wvtb = [persist.tile([P, NKO], BF16, tag=f"wvtb{i}", name=f"wvtb{i}") for i in range(NHT)]

        # ------------- phase 1: loads (chunked), QKV projections, RoPE ----
        with (
            tc.tile_pool(name="p1", bufs=2) as p1,
            tc.tile_pool(name="p1x", bufs=6) as p1x,
            tc.tile_pool(name="ps1", bufs=3, space="PSUM") as ps1,
            tc.tile_pool(name="ps1b", bufs=2, space="PSUM") as ps1b,
        ):
            # weights: SWDGE (gpsimd) DMA, ACT casts
            for i in range(NHT):
                wqs = p1.tile([P, NQO], F32, tag="wqstage")
                nc.gpsimd.dma_start(wqs[:], wqt_t[i])
                nc.scalar.copy(wqtb[i][:], wqs[:])
                wks = p1.tile([P, NKO], F32, tag="wkstage")
                nc.gpsimd.dma_start(wks[:], wkt_t[i])
                nc.scalar.copy(wktb[i][:], wks[:])
                wvs = p1.tile([P, NKO], F32, tag="wvstage")
                nc.gpsimd.dma_start(wvs[:], wvt_t[i])
                nc.scalar.copy(wvtb[i][:], wvs[:])

            xtbc = [[p1.tile([P, QC], BF16, tag=f"xtbc{i}_{sc}", name=f"xtbc{i}_{sc}",
                             bufs=1) for sc in range(NSC)] for i in range(NHT)]

            def rope_tile(dst_ap, ps, sc):
                """RoPE: dst = raw*C2 + (R @ raw)*SS for one [128, 512] chunk."""
                ssl = slice(QC * sc, QC * (sc + 1))
                raw = p1.tile([P, QC], BF16, tag="rope_raw")
                nc.scalar.copy(raw[:], ps[:])
                rps = ps1b.tile([P, QC], F32, tag="rps")
                nc.tensor.matmul(rps[:], lhsT=rot_sb[:], rhs=raw[:],
                                 start=True, stop=True)
                t1 = p1.tile([P, QC], F32, tag="rope_t1")
                nc.vector.tensor_mul(t1[:], raw[:], c2_sb[:, ssl])
                t2 = p1.tile([P, QC], F32, tag="rope_t2")
                nc.vector.tensor_mul(t2[:], rps[:], ss_sb[:, ssl])
                nc.vector.tensor_add(dst_ap, t1[:], t2[:])

            for sc in range(NSC):
                # X column-chunk loads (sync HWDGE) + DVE casts
                for i in range(NHT):
                    xs = p1x.tile([P, QC], F32, tag="xstage")
                    nc.sync.dma_start(xs[:], xt_t[i][:, QC * sc:QC * (sc + 1)])
                    nc.vector.tensor_copy(xtbc[i][sc][:], xs[:])
                # Q^T chunks (4 tiles x this chunk)
                for t in range(4):
                    ps = ps1.tile([P, QC], F32, tag="mmps")
                    for i in range(NHT):
                        nc.tensor.matmul(
                            ps[:], lhsT=wqtb[i][:, P * t:P * (t + 1)],
                            rhs=xtbc[i][sc][:],
                            start=(i == 0), stop=(i == NHT - 1),
                        )
                    rope_tile(qtbc[t][sc][:], ps, sc)
                # K^T chunk
                ps = ps1.tile([P, QC], F32, tag="mmps")
                for i in range(NHT):
                    nc.tensor.matmul(
                        ps[:], lhsT=wktb[i][:], rhs=xtbc[i][sc][:],
                        start=(i == 0), stop=(i == NHT - 1),
                    )
                rope_tile(ktbc[sc][:], ps, sc)
                # V tiles in this chunk
                for j in range(4 * sc, 4 * sc + 4):
                    jj = j - 4 * sc
                    ps = ps1b.tile([P, NKO], F32, tag="vps")
                    for i in range(NHT):
                        nc.tensor.matmul(
                            ps[:], lhsT=xtbc[i][sc][:, P * jj:P * (jj + 1)],
                            rhs=wvtb[i][:],
                            start=(i == 0), stop=(i == NHT - 1),
                        )
                    nc.scalar.copy(vaug[j][:, 0:64], ps[:, 0:64])
                    nc.scalar.copy(vaug[j][:, 65:129], ps[:, 64:128])
                    nc.gpsimd.memset(vaug[j][:, 64:65], 1.0)
                    nc.gpsimd.memset(vaug[j][:, 129:130], 1.0)

            # o_proj weights (needed last)
            for t in range(4):
                ws = p1x.tile([P, S], F32, tag="wostage", bufs=2)
                nc.gpsimd.dma_start(ws[:], wot_t[t])
                nc.scalar.copy(wotb[t][:], ws[:])

        # ------------- phase 2+3: attention (head-paired) + o_proj --------
        with (
            tc.tile_pool(name="p2", bufs=8) as p2,
            tc.tile_pool(name="p2a", bufs=3) as p2a,
            tc.tile_pool(name="p3", bufs=4) as p3,
            tc.tile_pool(name="ps2", bufs=4, space="PSUM") as ps2,
        ):
            def attn_scores(hp, off, qc, ki, avp, last):
                """scores + exp + AV for one head (off=0 or 64) and k-tile."""
                g = off // 64
                j = ki - 4 * qc
                col0 = P * j if j >= 0 else 0
                sp = ps2.tile([P, QC], F32, tag="sp", bufs=4)
                nc.tensor.matmul(
                    sp[:, col0:QC],
                    lhsT=ktbc[ki // 4][off:off + 64, P * (ki % 4):P * (ki % 4 + 1)],
                    rhs=qtbc[hp][qc][off:off + 64, col0:QC],
                    start=True, stop=True,
                )
                if j >= 0:
                    nc.vector.tensor_add(sp[:, col0:col0 + P],
                                         sp[:, col0:col0 + P], msk_sb[:])
                ep = p2.tile([P, QC], BF16, tag="ep")
                if col0 > 0:
                    nc.gpsimd.memset(ep[:, 0:col0], 0.0)
                nc.scalar.activation(ep[:, col0:QC], sp[:, col0:QC],
                                     AF.Exp, scale=EXP_SCALE)
                nc.tensor.matmul(
                    avp[:], lhsT=vaug[ki][:, 65 * g:65 * g + 65], rhs=ep[:],
                    start=(ki == 0), stop=last,
                )

            def normalize(hp, off, qc, avp):
                atr = p2a.tile([65, QC], F32, tag="atr")
                nc.vector.tensor_copy(atr[:], avp[:])
                nc.vector.reciprocal(atr[64:65, :], atr[64:65, :])
                rbc = ps2.tile([64, QC], F32, tag="rbc_op", bufs=2)
                nc.tensor.matmul(rbc[:], lhsT=ones65[64:65, :],
                                 rhs=atr[64:65, :], start=True, stop=True)
                nc.vector.tensor_mul(atbc[hp][qc][off:off + 64, :],
                                     atr[0:64, :], rbc[:])

            for qc in range(NSC):
                nkt = 4 * qc + 4
                for hp in range(4):
                    avpA = ps2.tile([65, QC], F32, tag="avp", bufs=2, name="avpA")
                    avpB = ps2.tile([65, QC], F32, tag="avp", bufs=2, name="avpB")
                    for ki in range(nkt):
                        last = ki == nkt - 1
                        attn_scores(hp, 0, qc, ki, avpA, last)
                        attn_scores(hp, 64, qc, ki, avpB, last)
                    normalize(hp, 0, qc, avpA)
                    normalize(hp, 64, qc, avpB)
                # o_proj for the s-tiles of this q-chunk
                for st in range(4 * qc, 4 * qc + 4):
                    stj = st - 4 * qc
                    for oc in range(NSC):
                        op = ps2.tile([P, QC], F32, tag="rbc_op", bufs=2, name="op")
                        for ft in range(4):
                            nc.tensor.matmul(
                                op[:],
                                lhsT=atbc[ft][qc][:, P * stj:P * (stj + 1)],
                                rhs=wotb[ft][:, QC * oc:QC * (oc + 1)],
                                start=(ft == 0), stop=(ft == 3),
                            )
                        ost = p3.tile([P, QC], F32, tag="ost")
                        nc.vector.tensor_copy(ost[:], op[:])
                        nc.sync.dma_start(y_t[st][:, QC * oc:QC * (oc + 1)], ost[:])

    nc.compile()
    return nc


def _host_tables():
    inv_freq = 1.0 / (10000.0 ** (np.arange(0, HD, 2, dtype=np.float32) / HD))
    pos = np.arange(S, dtype=np.float32)
    freqs = np.einsum('s,d->sd', pos, inv_freq)          # [S, 32]
    emb = np.concatenate([freqs, freqs], axis=-1)        # [S, 64]
    cosT = np.cos(emb).T.astype(np.float32)              # [64, S]
    sinT = np.sin(emb).T.astype(np.float32)
    c2 = np.ascontiguousarray(np.vstack([cosT, cosT]))   # [128, S]
    # sign of rotate_half is encoded in the rot matrix below; ss is plain sin
    ss = np.ascontiguousarray(np.vstack([sinT, sinT]))
    # rotate-half as a matmul: out[d] = sum_d' R[d', d] * in[d']
    R64 = np.zeros((HD, HD), dtype=np.float32)
    for d in range(32):
        R64[d + 32, d] = -1.0       # out[d] = -in[d+32]
        R64[d, d + 32] = 1.0        # out[d+32] = in[d]
    rot = np.zeros((P, P), dtype=np.float32)
    rot[0:64, 0:64] = R64
    rot[64:128, 64:128] = R64
    # causal bias for a diagonal 128x128 tile in scores^T[k, q] layout
    kk = np.arange(P)[:, None]
    qq = np.arange(P)[None, :]
    msk = np.where(kk <= qq, 0.0, MASK_VAL).astype(np.float32)
    import ml_dtypes
    rot = rot.astype(ml_dtypes.bfloat16)   # exact: entries are 0/+-1
    return c2, ss, rot, msk


# q/o head order within a rank block: pair heads (u, u+4) in each 128-row tile
_HEAD_ORDER = [0, 4, 1, 5, 2, 6, 3, 7]


def _make_in_maps(hidden_states, Wq, Wk, Wv, Wo):
    hs = np.ascontiguousarray(np.asarray(hidden_states, dtype=np.float32))
    Wq = np.asarray(Wq, dtype=np.float32)
    Wk = np.asarray(Wk, dtype=np.float32)
    Wv = np.asarray(Wv, dtype=np.float32)
    Wo = np.asarray(Wo, dtype=np.float32)
    c2, ss, rot, msk = _host_tables()
    in_maps = []
    for c in range(8):
        b, r = c // 4, c % 4
        # row indices of Wq (= cols of Wo) for this rank, in device head order
        qrows = np.concatenate([
            np.arange(HD) + (NHL * r + u) * HD for u in _HEAD_ORDER
        ])
        in_maps.append({
            "xt": np.ascontiguousarray(hs[b].T),
            "wqt": np.ascontiguousarray(Wq[qrows, :].T),
            "wkt": np.ascontiguousarray(Wk[NKO * r:NKO * (r + 1), :].T),
            "wvt": np.ascontiguousarray(Wv[NKO * r:NKO * (r + 1), :].T),
            "wot": np.ascontiguousarray(Wo[:, qrows].T),
            "c2": c2, "ss": ss, "msk": msk, "rot": rot,
        })
    return in_maps


_NC = None


def _get_nc():
    global _NC
    if _NC is None:
        _NC = build_nc()
    return _NC


def run_cores(hidden_states, Wq, Wk, Wv, Wo, **run_kwargs):
    """Run the SPMD kernel; returns (out [B,S,H] fp32, BassKernelResults)."""
    nc = _get_nc()
    in_maps = _make_in_maps(hidden_states, Wq, Wk, Wv, Wo)
    res = run_bass_kernel_spmd(nc, in_maps, list(range(8)), **run_kwargs)
    out = np.zeros((B, S, H), dtype=np.float32)
    for c in range(8):
        out[c // 4] += res.results[c]["y"]
    return out, res


def kernel(hidden_states, Wq, Wk, Wv, Wo):
    out, _ = run_cores(hidden_states, Wq, Wk, Wv, Wo)
    return out


# revision 11
# speedup vs baseline: 1.6640x; 1.2472x over previous
"""Trainium2 Bass kernel for GQA attention (B=2, S=2048, H=2048, 32 Q heads,
8 KV heads, HD=64, RoPE, causal) with output projection.

Sharding: TP=4 over heads within each batch, DP=2 over batch -> 8 cores.
Core c handles batch c//4, head-rank c%4 (8 Q heads, 2 KV heads).
Each core computes a partial o_proj output [S, H]; the host sums the 4
partials per batch (cheaper than on-device all-reduce at these sizes).

Per-core layout (all transposed layouts produced by host as pure re-indexing):
  xt  [H, S]   = hidden[b].T                  fp32
  wqt [H, 512] = Wq[head-block r].T           fp32  (head order h0,h4,h1,h5,h2,h6,h3,h7)
  wkt [H, 128] = Wk[kv-block r].T             fp32
  wvt [H, 128] = Wv[kv-block r].T             fp32
  wot [512, H] = Wo[:, block r].T             fp32  (rows in same head order as wqt)
  c2/ss [128, S] RoPE cos/sin tables (two 64-row head blocks stacked)
  rot [128, 128] rotate-half permutation (+/-1) matrix, msk [128,128] causal bias

On device: cast to bf16, Q^T/K^T/V projections on PE, RoPE via PE rotation
matmul + DVE, scores^T = K^T.T Q^T per head (causal-trimmed), exp on ACT
(no max subtraction; scores are bounded ~|6|), AV with ones-augmented V to
get softmax denominators for free, normalize, o_proj. fp32 PSUM throughout.
"""

import numpy as np
from contextlib import ExitStack

import concourse.bass as bass
import concourse.bacc as bacc
import concourse.mybir as mybir
import concourse.tile as tile
from concourse.bass_utils import run_bass_kernel_spmd

F32 = mybir.dt.float32
BF16 = mybir.dt.bfloat16
AF = mybir.ActivationFunctionType

B, S, H = 2, 2048, 2048
NH, NKV, HD = 32, 8, 64
TP = 4                      # head-parallel ranks per batch
NQO = NH // TP * HD         # 512 per-core q features (8 heads)
NKO = NKV // TP * HD        # 128 per-core kv features (2 heads)
NHL = NH // TP              # 8 local q heads
EXP_SCALE = 1.0 / 8.0       # 1/sqrt(HD)
MASK_VAL = -30000.0
P = 128
QC = 512                    # q-chunk (one PSUM bank of fp32)
NSC = S // QC               # 4 q/s chunks
NPT = S // P                # 16 partition tiles of S
NHT = H // P                # 16 partition tiles of H

# local head h (0..7) -> (q-tile index, partition offset, local kv head)
# q heads are stored pairing (t, t+4) so that the d-partition offset of the
# q head always equals the d-partition offset of its kv head (PE array rows
# must line up between lhsT and rhs).
def _head_pos(h):
    g = h // 4               # local kv head, also partition block of K^T
    return h % 4, 64 * g, g


def build_nc():
    nc = bacc.Bacc("TRN2", target_bir_lowering=False, debug=False, num_devices=8)

    xt = nc.dram_tensor("xt", [H, S], F32, kind="ExternalInput").ap()
    wqt = nc.dram_tensor("wqt", [H, NQO], F32, kind="ExternalInput").ap()
    wkt = nc.dram_tensor("wkt", [H, NKO], F32, kind="ExternalInput").ap()
    wvt = nc.dram_tensor("wvt", [H, NKO], F32, kind="ExternalInput").ap()
    wot = nc.dram_tensor("wot", [NQO, H], F32, kind="ExternalInput").ap()
    c2 = nc.dram_tensor("c2", [P, S], F32, kind="ExternalInput").ap()
    ss = nc.dram_tensor("ss", [P, S], F32, kind="ExternalInput").ap()
    msk = nc.dram_tensor("msk", [P, P], F32, kind="ExternalInput").ap()
    rot = nc.dram_tensor("rot", [P, P], BF16, kind="ExternalInput").ap()
    y = nc.dram_tensor("y", [S, H], F32, kind="ExternalOutput").ap()

    xt_t = xt.rearrange("(n p) s -> n p s", p=P)
    wqt_t = wqt.rearrange("(n p) o -> n p o", p=P)
    wkt_t = wkt.rearrange("(n p) o -> n p o", p=P)
    wvt_t = wvt.rearrange("(n p) o -> n p o", p=P)
    wot_t = wot.rearrange("(n p) o -> n p o", p=P)
    y_t = y.rearrange("(n p) o -> n p o", p=P)

    with tile.TileContext(nc) as tc, ExitStack() as ctx:
        persist = ctx.enter_context(tc.tile_pool(name="persist", bufs=1))

        c2_sb = persist.tile([P, S], F32, tag="c2", name="c2sb")
        ss_sb = persist.tile([P, S], F32, tag="ss", name="sssb")
        msk_sb = persist.tile([P, P], F32, tag="msk", name="msksb")
        rot_sb = persist.tile([P, P], BF16, tag="rot", name="rotsb")
        ones65 = persist.tile([65, 64], F32, tag="ones65", name="ones65")
        nc.gpsimd.dma_start(c2_sb[:], c2[:])
        nc.gpsimd.dma_start(ss_sb[:], ss[:])
        nc.gpsimd.dma_start(msk_sb[:], msk[:])
        nc.gpsimd.dma_start(rot_sb[:], rot[:])
        nc.gpsimd.memset(ones65[64:65, :], 1.0)

        # chunked persistent activations: [tile][chunk] of [128, 512]
        qtbc = [[persist.tile([P, QC], BF16, tag=f"qtbc{t}_{sc}", name=f"qtbc{t}_{sc}")
                 for sc in range(NSC)] for t in range(4)]
        ktbc = [persist.tile([P, QC], BF16, tag=f"ktbc{sc}", name=f"ktbc{sc}")
                for sc in range(NSC)]
        vaug = [persist.tile([P, 130], BF16, tag=f"vaug{i}", name=f"vaug{i}")
                for i in range(NPT)]
        atbc = [[persist.tile([P, QC], BF16, tag=f"atbc{t}_{qc}", name=f"atbc{t}_{qc}")
                 for qc in range(NSC)] for t in range(4)]
        wotb = [persist.tile([P, S], BF16, tag=f"wotb{t}", name=f"wotb{t}") for t in range(4)]
        wqtb = [persist.tile([P, NQO], BF16, tag=f"wqtb{i}", name=f"wqtb{i}") for i in range(NHT)]
        wktb = [persist.tile([P, NKO], BF16, tag=f"wktb{i}", name=f"wktb{i}") for i in range(NHT)]
        wvtb = [persist.tile([P, NKO], BF16, tag=f"wvtb{i}", name=f"wvtb{i}") for i in range(NHT)]

        # ------------- phase 1: loads (chunked), QKV projections, RoPE ----
        with (
            tc.tile_pool(name="p1", bufs=2) as p1,
            tc.tile_pool(name="p1x", bufs=6) as p1x,
            tc.tile_pool(name="ps1", bufs=3, space="PSUM") as ps1,
            tc.tile_pool(name="ps1b", bufs=2, space="PSUM") as ps1b,
        ):
            # weights: SWDGE (gpsimd) DMA, ACT casts
            for i in range(NHT):
                wqs = p1.tile([P, NQO], F32, tag="wqstage")
                nc.gpsimd.dma_start(wqs[:], wqt_t[i])
                nc.scalar.copy(wqtb[i][:], wqs[:])
                wks = p1.tile([P, NKO], F32, tag="wkstage")
                nc.gpsimd.dma_start(wks[:], wkt_t[i])
                nc.scalar.copy(wktb[i][:], wks[:])
                wvs = p1.tile([P, NKO], F32, tag="wvstage")
                nc.gpsimd.dma_start(wvs[:], wvt_t[i])
                nc.scalar.copy(wvtb[i][:], wvs[:])

            xtbc = [[p1.tile([P, QC], BF16, tag=f"xtbc{i}_{sc}", name=f"xtbc{i}_{sc}",
                             bufs=1) for sc in range(NSC)] for i in range(NHT)]

            def rope_tile(dst_ap, ps, sc):
                """RoPE: dst = raw*C2 + (R @ raw)*SS for one [128, 512] chunk."""
                ssl = slice(QC * sc, QC * (sc + 1))
                raw = p1.tile([P, QC], BF16, tag="rope_raw")
                nc.scalar.copy(raw[:], ps[:])
                rps = ps1b.tile([P, QC], F32, tag="rps")
                nc.tensor.matmul(rps[:], lhsT=rot_sb[:], rhs=raw[:],
                                 start=True, stop=True)
                t1 = p1.tile([P, QC], F32, tag="rope_t1")
                nc.vector.tensor_mul(t1[:], raw[:], c2_sb[:, ssl])
                t2 = p1.tile([P, QC], F32, tag="rope_t2")
                nc.vector.tensor_mul(t2[:], rps[:], ss_sb[:, ssl])
                nc.vector.tensor_add(dst_ap, t1[:], t2[:])

            for sc in range(NSC):
                # X column-chunk loads (sync HWDGE) + DVE casts
                for i in range(NHT):
                    xs = p1x.tile([P, QC], F32, tag="xstage")
                    deng = nc.sync if i % 2 == 0 else nc.gpsimd
                    deng.dma_start(xs[:], xt_t[i][:, QC * sc:QC * (sc + 1)])
                    nc.vector.tensor_copy(xtbc[i][sc][:], xs[:])
                # Q^T chunks (4 tiles x this chunk)
                for t in range(4):
                    ps = ps1.tile([P, QC], F32, tag="mmps")
                    for i in range(NHT):
                        nc.tensor.matmul(
                            ps[:], lhsT=wqtb[i][:, P * t:P * (t + 1)],
                            rhs=xtbc[i][sc][:],
                            start=(i == 0), stop=(i == NHT - 1),
                        )
                    rope_tile(qtbc[t][sc][:], ps, sc)
                # K^T chunk
                ps = ps1.tile([P, QC], F32, tag="mmps")
                for i in range(NHT):
                    nc.tensor.matmul(
                        ps[:], lhsT=wktb[i][:], rhs=xtbc[i][sc][:],
                        start=(i == 0), stop=(i == NHT - 1),
                    )
                rope_tile(ktbc[sc][:], ps, sc)
                # V tiles in this chunk
                for j in range(4 * sc, 4 * sc + 4):
                    jj = j - 4 * sc
                    ps = ps1b.tile([P, NKO], F32, tag="vps")
                    for i in range(NHT):
                        nc.tensor.matmul(
                            ps[:], lhsT=xtbc[i][sc][:, P * jj:P * (jj + 1)],
                            rhs=wvtb[i][:],
                            start=(i == 0), stop=(i == NHT - 1),
                        )
                    nc.scalar.copy(vaug[j][:, 0:64], ps[:, 0:64])
                    nc.scalar.copy(vaug[j][:, 65:129], ps[:, 64:128])
                    nc.gpsimd.memset(vaug[j][:, 64:65], 1.0)
                    nc.gpsimd.memset(vaug[j][:, 129:130], 1.0)

            # o_proj weights (needed last)
            for t in range(4):
                ws = p1x.tile([P, S], F32, tag="wostage", bufs=2)
                nc.gpsimd.dma_start(ws[:], wot_t[t])
                nc.scalar.copy(wotb[t][:], ws[:])

        # ------------- phase 2+3: attention (head-paired) + o_proj --------
        with (
            tc.tile_pool(name="p2", bufs=8) as p2,
            tc.tile_pool(name="p2a", bufs=4) as p2a,
            tc.tile_pool(name="p3", bufs=4) as p3,
            tc.tile_pool(name="ps2", bufs=4, space="PSUM") as ps2,
        ):
            def attn_scores(hp, off, qc, ki, avp, last):
                """scores + exp + AV for one head (off=0 or 64) and k-tile."""
                g = off // 64
                j = ki - 4 * qc
                col0 = P * j if j >= 0 else 0
                sp = ps2.tile([P, QC], F32, tag="sp", bufs=4)
                nc.tensor.matmul(
                    sp[:, col0:QC],
                    lhsT=ktbc[ki // 4][off:off + 64, P * (ki % 4):P * (ki % 4 + 1)],
                    rhs=qtbc[hp][qc][off:off + 64, col0:QC],
                    start=True, stop=True,
                )
                if j >= 0:
                    nc.vector.tensor_add(sp[:, col0:col0 + P],
                                         sp[:, col0:col0 + P], msk_sb[:])
                ep = p2.tile([P, QC], BF16, tag="ep")
                if col0 > 0:
                    nc.gpsimd.memset(ep[:, 0:col0], 0.0)
                nc.scalar.activation(ep[:, col0:QC], sp[:, col0:QC],
                                     AF.Exp, scale=EXP_SCALE)
                nc.tensor.matmul(
                    avp[:], lhsT=vaug[ki][:, 65 * g:65 * g + 65], rhs=ep[:],
                    start=(ki == 0), stop=last,
                )

            def normalize(hp, off, qc, avp):
                # 1/rowsum = exp(-ln(rowsum)): both fns live in the same ACT
                # table set as the softmax exp, so no table switches.
                lns = p2a.tile([65, QC], F32, tag="lns")
                nc.scalar.activation(lns[64:65, :], avp[64:65, :], AF.Ln)
                rcp = p2a.tile([65, QC], F32, tag="rcp")
                nc.scalar.activation(rcp[64:65, :], lns[64:65, :], AF.Exp,
                                     scale=-1.0)
                rbc = ps2.tile([64, QC], F32, tag="sp", bufs=4, name="rbc")
                nc.tensor.matmul(rbc[:], lhsT=ones65[64:65, 0:64],
                                 rhs=rcp[64:65, :], start=True, stop=True)
                atrs = p2a.tile([64, QC], F32, tag="atrs")
                nc.vector.tensor_copy(atrs[:], avp[0:64, :])
                nc.vector.tensor_mul(atbc[hp][qc][off:off + 64, :],
                                     atrs[:], rbc[:])

            def oproj_piece(qc, st):
                stj = st - 4 * qc
                for oc in range(NSC):
                    op = ps2.tile([P, QC], F32, tag="sp", bufs=4, name="op")
                    for ft in range(4):
                        nc.tensor.matmul(
                            op[:],
                            lhsT=atbc[ft][qc][:, P * stj:P * (stj + 1)],
                            rhs=wotb[ft][:, QC * oc:QC * (oc + 1)],
                            start=(ft == 0), stop=(ft == 3),
                        )
                    ost = p3.tile([P, QC], F32, tag="ost")
                    nc.vector.tensor_copy(ost[:], op[:])
                    nc.sync.dma_start(y_t[st][:, QC * oc:QC * (oc + 1)], ost[:])

            from collections import deque
            oproj_q = deque()
            pending_norms = []
            for qc in range(NSC):
                nkt = 4 * qc + 4
                for hp in range(4):
                    avpA = ps2.tile([65, QC], F32, tag="avp", bufs=4, name="avpA")
                    avpB = ps2.tile([65, QC], F32, tag="avp", bufs=4, name="avpB")
                    for ki in range(nkt):
                        last = ki == nkt - 1
                        attn_scores(hp, 0, qc, ki, avpA, last)
                        attn_scores(hp, 64, qc, ki, avpB, last)
                    # flush deferred work from the previous group to fill the
                    # PE pipeline while this group's exps/AVs drain
                    prev, pending_norms = pending_norms, [
                        (hp, 0, qc, avpA), (hp, 64, qc, avpB)]
                    for args in prev:
                        normalize(*args)
                    if oproj_q:
                        oproj_piece(*oproj_q.popleft())
                for st in range(4 * qc, 4 * qc + 4):
                    oproj_q.append((qc, st))
            for args in pending_norms:
                normalize(*args)
            while oproj_q:
                oproj_piece(*oproj_q.popleft())

    nc.compile()
    return nc


def _host_tables():
    inv_freq = 1.0 / (10000.0 ** (np.arange(0, HD, 2, dtype=np.float32) / HD))
    pos = np.arange(S, dtype=np.float32)
    freqs = np.einsum('s,d->sd', pos, inv_freq)          # [S, 32]
    emb = np.concatenate([freqs, freqs], axis=-1)        # [S, 64]
    cosT = np.cos(emb).T.astype(np.float32)              # [64, S]
    sinT = np.sin(emb).T.astype(np.float32)
    c2 = np.ascontiguousarray(np.vstack([cosT, cosT]))   # [128, S]
    # sign of rotate_half is encoded in the rot matrix below; ss is plain sin
    ss = np.ascontiguousarray(np.vstack([sinT, sinT]))
    # rotate-half as a matmul: out[d] = sum_d' R[d', d] * in[d']
    R64 = np.zeros((HD, HD), dtype=np.float32)
    for d in range(32):
        R64[d + 32, d] = -1.0       # out[d] = -in[d+32]
        R64[d, d + 32] = 1.0        # out[d+32] = in[d]
    rot = np.zeros((P, P), dtype=np.float32)
    rot[0:64, 0:64] = R64
    rot[64:128, 64:128] = R64
    # causal bias for a diagonal 128x128 tile in scores^T[k, q] layout
    kk = np.arange(P)[:, None]
    qq = np.arange(P)[None, :]
    msk = np.where(kk <= qq, 0.0, MASK_VAL).astype(np.float32)
    import ml_dtypes
    rot = rot.astype(ml_dtypes.bfloat16)   # exact: entries are 0/+-1
    return c2, ss, rot, msk


# q/o head order within a rank block: pair heads (u, u+4) in each 128-row tile
_HEAD_ORDER = [0, 4, 1, 5, 2, 6, 3, 7]


def _make_in_maps(hidden_states, Wq, Wk, Wv, Wo):
    hs = np.ascontiguousarray(np.asarray(hidden_states, dtype=np.float32))
    Wq = np.asarray(Wq, dtype=np.float32)
    Wk = np.asarray(Wk, dtype=np.float32)
    Wv = np.asarray(Wv, dtype=np.float32)
    Wo = np.asarray(Wo, dtype=np.float32)
    c2, ss, rot, msk = _host_tables()
    in_maps = []
    for c in range(8):
        b, r = c // 4, c % 4
        # row indices of Wq (= cols of Wo) for this rank, in device head order
        qrows = np.concatenate([
            np.arange(HD) + (NHL * r + u) * HD for u in _HEAD_ORDER
        ])
        in_maps.append({
            "xt": np.ascontiguousarray(hs[b].T),
            "wqt": np.ascontiguousarray(Wq[qrows, :].T),
            "wkt": np.ascontiguousarray(Wk[NKO * r:NKO * (r + 1), :].T),
            "wvt": np.ascontiguousarray(Wv[NKO * r:NKO * (r + 1), :].T),
            "wot": np.ascontiguousarray(Wo[:, qrows].T),
            "c2": c2, "ss": ss, "msk": msk, "rot": rot,
        })
    return in_maps


_NC = None


def _get_nc():
    global _NC
    if _NC is None:
        _NC = build_nc()
    return _NC


def run_cores(hidden_states, Wq, Wk, Wv, Wo, **run_kwargs):
    """Run the SPMD kernel; returns (out [B,S,H] fp32, BassKernelResults)."""
    nc = _get_nc()
    in_maps = _make_in_maps(hidden_states, Wq, Wk, Wv, Wo)
    res = run_bass_kernel_spmd(nc, in_maps, list(range(8)), **run_kwargs)
    out = np.zeros((B, S, H), dtype=np.float32)
    for c in range(8):
        out[c // 4] += res.results[c]["y"]
    return out, res


def kernel(hidden_states, Wq, Wk, Wv, Wo):
    out, _ = run_cores(hidden_states, Wq, Wk, Wv, Wo)
    return out


# revision 12
# speedup vs baseline: 1.6861x; 1.0133x over previous
"""Trainium2 Bass kernel for GQA attention (B=2, S=2048, H=2048, 32 Q heads,
8 KV heads, HD=64, RoPE, causal) with output projection.

Sharding: TP=4 over heads within each batch, DP=2 over batch -> 8 cores.
Core c handles batch c//4, head-rank c%4 (8 Q heads, 2 KV heads).
Each core computes a partial o_proj output [S, H]; the host sums the 4
partials per batch (cheaper than on-device all-reduce at these sizes).

Per-core layout (all transposed layouts produced by host as pure re-indexing):
  xt  [H, S]   = hidden[b].T                  fp32
  wqt [H, 512] = Wq[head-block r].T           fp32  (head order h0,h4,h1,h5,h2,h6,h3,h7)
  wkt [H, 128] = Wk[kv-block r].T             fp32
  wvt [H, 128] = Wv[kv-block r].T             fp32
  wot [512, H] = Wo[:, block r].T             fp32  (rows in same head order as wqt)
  c2/ss [128, S] RoPE cos/sin tables (two 64-row head blocks stacked)
  rot [128, 128] rotate-half permutation (+/-1) matrix, msk [128,128] causal bias

On device: cast to bf16, Q^T/K^T/V projections on PE, RoPE via PE rotation
matmul + DVE, scores^T = K^T.T Q^T per head (causal-trimmed), exp on ACT
(no max subtraction; scores are bounded ~|6|), AV with ones-augmented V to
get softmax denominators for free, normalize, o_proj. fp32 PSUM throughout.
"""

import numpy as np
from contextlib import ExitStack

import concourse.bass as bass
import concourse.bacc as bacc
import concourse.mybir as mybir
import concourse.tile as tile
from concourse.bass_utils import run_bass_kernel_spmd

F32 = mybir.dt.float32
BF16 = mybir.dt.bfloat16
AF = mybir.ActivationFunctionType

B, S, H = 2, 2048, 2048
NH, NKV, HD = 32, 8, 64
TP = 4                      # head-parallel ranks per batch
NQO = NH // TP * HD         # 512 per-core q features (8 heads)
NKO = NKV // TP * HD        # 128 per-core kv features (2 heads)
NHL = NH // TP              # 8 local q heads
EXP_SCALE = 1.0 / 8.0       # 1/sqrt(HD)
MASK_VAL = -30000.0
P = 128
QC = 512                    # q-chunk (one PSUM bank of fp32)
NSC = S // QC               # 4 q/s chunks
NPT = S // P                # 16 partition tiles of S
NHT = H // P                # 16 partition tiles of H

# local head h (0..7) -> (q-tile index, partition offset, local kv head)
# q heads are stored pairing (t, t+4) so that the d-partition offset of the
# q head always equals the d-partition offset of its kv head (PE array rows
# must line up between lhsT and rhs).
def _head_pos(h):
    g = h // 4               # local kv head, also partition block of K^T
    return h % 4, 64 * g, g


def build_nc():
    nc = bacc.Bacc("TRN2", target_bir_lowering=False, debug=False, num_devices=8)

    xt = nc.dram_tensor("xt", [H, S], F32, kind="ExternalInput").ap()
    wqt = nc.dram_tensor("wqt", [H, NQO], F32, kind="ExternalInput").ap()
    wkt = nc.dram_tensor("wkt", [H, NKO], F32, kind="ExternalInput").ap()
    wvt = nc.dram_tensor("wvt", [H, NKO], F32, kind="ExternalInput").ap()
    wot = nc.dram_tensor("wot", [NQO, H], F32, kind="ExternalInput").ap()
    c2 = nc.dram_tensor("c2", [P, S], F32, kind="ExternalInput").ap()
    ss = nc.dram_tensor("ss", [P, S], F32, kind="ExternalInput").ap()
    msk = nc.dram_tensor("msk", [P, P], F32, kind="ExternalInput").ap()
    rot = nc.dram_tensor("rot", [P, P], BF16, kind="ExternalInput").ap()
    y = nc.dram_tensor("y", [S, H], F32, kind="ExternalOutput").ap()

    xt_t = xt.rearrange("(n p) s -> n p s", p=P)
    wqt_t = wqt.rearrange("(n p) o -> n p o", p=P)
    wkt_t = wkt.rearrange("(n p) o -> n p o", p=P)
    wvt_t = wvt.rearrange("(n p) o -> n p o", p=P)
    wot_t = wot.rearrange("(n p) o -> n p o", p=P)
    y_t = y.rearrange("(n p) o -> n p o", p=P)

    with tile.TileContext(nc) as tc, ExitStack() as ctx:
        persist = ctx.enter_context(tc.tile_pool(name="persist", bufs=1))

        c2_sb = persist.tile([P, S], F32, tag="c2", name="c2sb")
        ss_sb = persist.tile([P, S], F32, tag="ss", name="sssb")
        msk_sb = persist.tile([P, P], F32, tag="msk", name="msksb")
        rot_sb = persist.tile([P, P], BF16, tag="rot", name="rotsb")
        ones65 = persist.tile([65, 64], F32, tag="ones65", name="ones65")
        ones65b = persist.tile([65, 64], BF16, tag="ones65b", name="ones65b")
        nc.gpsimd.dma_start(c2_sb[:], c2[:])
        nc.gpsimd.dma_start(ss_sb[:], ss[:])
        nc.gpsimd.dma_start(msk_sb[:], msk[:])
        nc.gpsimd.dma_start(rot_sb[:], rot[:])
        nc.gpsimd.memset(ones65[64:65, :], 1.0)
        nc.gpsimd.memset(ones65b[64:65, :], 1.0)

        # chunked persistent activations: [tile][chunk] of [128, 512]
        qtbc = [[persist.tile([P, QC], BF16, tag=f"qtbc{t}_{sc}", name=f"qtbc{t}_{sc}")
                 for sc in range(NSC)] for t in range(4)]
        ktbc = [persist.tile([P, QC], BF16, tag=f"ktbc{sc}", name=f"ktbc{sc}")
                for sc in range(NSC)]
        vaug = [persist.tile([P, 130], BF16, tag=f"vaug{i}", name=f"vaug{i}")
                for i in range(NPT)]
        atbc = [[persist.tile([P, QC], BF16, tag=f"atbc{t}_{qc}", name=f"atbc{t}_{qc}")
                 for qc in range(NSC)] for t in range(4)]
        wotb = [persist.tile([P, S], BF16, tag=f"wotb{t}", name=f"wotb{t}") for t in range(4)]
        wqtb = [persist.tile([P, NQO], BF16, tag=f"wqtb{i}", name=f"wqtb{i}") for i in range(NHT)]
        wktb = [persist.tile([P, NKO], BF16, tag=f"wktb{i}", name=f"wktb{i}") for i in range(NHT)]
        wvtb = [persist.tile([P, NKO], BF16, tag=f"wvtb{i}", name=f"wvtb{i}") for i in range(NHT)]

        # ------------- phase 1: loads (chunked), QKV projections, RoPE ----
        with (
            tc.tile_pool(name="p1", bufs=2) as p1,
            tc.tile_pool(name="p1x", bufs=6) as p1x,
            tc.tile_pool(name="ps1", bufs=3, space="PSUM") as ps1,
            tc.tile_pool(name="ps1b", bufs=2, space="PSUM") as ps1b,
        ):
            # weights: SWDGE (gpsimd) DMA, ACT casts
            for i in range(NHT):
                wqs = p1.tile([P, NQO], F32, tag="wqstage")
                nc.gpsimd.dma_start(wqs[:], wqt_t[i])
                nc.scalar.copy(wqtb[i][:], wqs[:])
                wks = p1.tile([P, NKO], F32, tag="wkstage")
                nc.gpsimd.dma_start(wks[:], wkt_t[i])
                nc.scalar.copy(wktb[i][:], wks[:])
                wvs = p1.tile([P, NKO], F32, tag="wvstage")
                nc.gpsimd.dma_start(wvs[:], wvt_t[i])
                nc.scalar.copy(wvtb[i][:], wvs[:])

            xtbc = [[p1.tile([P, QC], BF16, tag=f"xtbc{i}_{sc}", name=f"xtbc{i}_{sc}",
                             bufs=1) for sc in range(NSC)] for i in range(NHT)]

            def rope_tile(dst_ap, ps, sc):
                """RoPE: dst = raw*C2 + (R @ raw)*SS for one [128, 512] chunk."""
                ssl = slice(QC * sc, QC * (sc + 1))
                raw = p1.tile([P, QC], BF16, tag="rope_raw")
                nc.scalar.copy(raw[:], ps[:])
                rps = ps1b.tile([P, QC], F32, tag="rps")
                nc.tensor.matmul(rps[:], lhsT=rot_sb[:], rhs=raw[:],
                                 start=True, stop=True)
                t1 = p1.tile([P, QC], F32, tag="rope_t1")
                nc.vector.tensor_mul(t1[:], raw[:], c2_sb[:, ssl])
                t2 = p1.tile([P, QC], F32, tag="rope_t2")
                nc.vector.tensor_mul(t2[:], rps[:], ss_sb[:, ssl])
                nc.vector.tensor_add(dst_ap, t1[:], t2[:])

            for sc in range(NSC):
                # X column-chunk loads (sync HWDGE) + DVE casts
                for i in range(NHT):
                    xs = p1x.tile([P, QC], F32, tag="xstage")
                    deng = nc.sync if i % 2 == 0 else nc.gpsimd
                    deng.dma_start(xs[:], xt_t[i][:, QC * sc:QC * (sc + 1)])
                    nc.vector.tensor_copy(xtbc[i][sc][:], xs[:])
                # Q^T chunks (4 tiles x this chunk)
                for t in range(4):
                    ps = ps1.tile([P, QC], F32, tag="mmps")
                    for i in range(NHT):
                        nc.tensor.matmul(
                            ps[:], lhsT=wqtb[i][:, P * t:P * (t + 1)],
                            rhs=xtbc[i][sc][:],
                            start=(i == 0), stop=(i == NHT - 1),
                        )
                    rope_tile(qtbc[t][sc][:], ps, sc)
                # K^T chunk
                ps = ps1.tile([P, QC], F32, tag="mmps")
                for i in range(NHT):
                    nc.tensor.matmul(
                        ps[:], lhsT=wktb[i][:], rhs=xtbc[i][sc][:],
                        start=(i == 0), stop=(i == NHT - 1),
                    )
                rope_tile(ktbc[sc][:], ps, sc)
                # V tiles in this chunk
                for j in range(4 * sc, 4 * sc + 4):
                    jj = j - 4 * sc
                    ps = ps1b.tile([P, NKO], F32, tag="vps")
                    for i in range(NHT):
                        nc.tensor.matmul(
                            ps[:], lhsT=xtbc[i][sc][:, P * jj:P * (jj + 1)],
                            rhs=wvtb[i][:],
                            start=(i == 0), stop=(i == NHT - 1),
                        )
                    nc.vector.tensor_copy(vaug[j][:, 0:64], ps[:, 0:64])
                    nc.vector.tensor_copy(vaug[j][:, 65:129], ps[:, 64:128])
                    nc.gpsimd.memset(vaug[j][:, 64:65], 1.0)
                    nc.gpsimd.memset(vaug[j][:, 129:130], 1.0)

            # o_proj weights (needed last)
            for t in range(4):
                ws = p1x.tile([P, S], F32, tag="wostage", bufs=2)
                nc.gpsimd.dma_start(ws[:], wot_t[t])
                nc.scalar.copy(wotb[t][:], ws[:])

        # ------------- phase 2+3: attention (head-paired) + o_proj --------
        with (
            tc.tile_pool(name="p2", bufs=8) as p2,
            tc.tile_pool(name="p2a", bufs=4) as p2a,
            tc.tile_pool(name="p3", bufs=4) as p3,
            tc.tile_pool(name="ps2", bufs=4, space="PSUM") as ps2,
        ):
            def attn_step(hp, qc, ki, avpA, avpB, last):
                """Both heads of the pair share one 2-bank score tile so the
                exp runs as a single wide ACT op (ACT overhead dominates)."""
                j = ki - 4 * qc
                col0 = P * j if j >= 0 else 0
                kc = P * (ki % 4)
                sp = ps2.tile([P, 2 * QC], F32, tag="sp", bufs=2)
                nc.tensor.matmul(
                    sp[:, col0:QC],
                    lhsT=ktbc[ki // 4][0:64, kc:kc + P],
                    rhs=qtbc[hp][qc][0:64, col0:QC],
                    start=True, stop=True,
                )
                nc.tensor.matmul(
                    sp[:, QC + col0:2 * QC],
                    lhsT=ktbc[ki // 4][64:128, kc:kc + P],
                    rhs=qtbc[hp][qc][64:128, col0:QC],
                    start=True, stop=True,
                )
                ep = p2.tile([P, 2 * QC], BF16, tag="ep")
                if j >= 0:
                    nc.vector.tensor_add(sp[:, col0:col0 + P],
                                         sp[:, col0:col0 + P], msk_sb[:])
                    nc.vector.tensor_add(sp[:, QC + col0:QC + col0 + P],
                                         sp[:, QC + col0:QC + col0 + P], msk_sb[:])
                    if col0 > 0:
                        nc.gpsimd.memset(ep[:, 0:col0], 0.0)
                        nc.gpsimd.memset(ep[:, QC:QC + col0], 0.0)
                    nc.scalar.activation(ep[:, col0:QC], sp[:, col0:QC],
                                         AF.Exp, scale=EXP_SCALE)
                    nc.scalar.activation(ep[:, QC + col0:2 * QC],
                                         sp[:, QC + col0:2 * QC],
                                         AF.Exp, scale=EXP_SCALE)
                else:
                    nc.scalar.activation(ep[:], sp[:], AF.Exp, scale=EXP_SCALE)
                nc.tensor.matmul(
                    avpA[:], lhsT=vaug[ki][:, 0:65], rhs=ep[:, 0:QC],
                    start=(ki == 0), stop=last,
                )
                nc.tensor.matmul(
                    avpB[:], lhsT=vaug[ki][:, 65:130], rhs=ep[:, QC:2 * QC],
                    start=(ki == 0), stop=last,
                )

            def normalize(hp, off, qc, avp):
                # 1/rowsum = exp(-ln(rowsum)): both fns live in the same ACT
                # table set as the softmax exp, so no table switches.
                lns = p2a.tile([65, QC], F32, tag="lns")
                nc.scalar.activation(lns[64:65, :], avp[64:65, :], AF.Ln)
                rcp = p2a.tile([65, QC], BF16, tag="rcp")
                nc.scalar.activation(rcp[64:65, :], lns[64:65, :], AF.Exp,
                                     scale=-1.0)
                atrs = p2a.tile([64, QC], F32, tag="atrs")
                nc.vector.tensor_copy(atrs[:], avp[0:64, :])
                rbc = ps2.tile([64, QC], F32, tag="avp", bufs=4, name="rbc")
                nc.tensor.matmul(rbc[:], lhsT=ones65b[64:65, 0:64],
                                 rhs=rcp[64:65, :], start=True, stop=True)
                nc.vector.tensor_mul(atbc[hp][qc][off:off + 64, :],
                                     atrs[:], rbc[:])

            def oproj_piece(qc, st):
                stj = st - 4 * qc
                for oc in range(NSC):
                    op = ps2.tile([P, QC], F32, tag="sp", bufs=2, name="op")
                    for ft in range(4):
                        nc.tensor.matmul(
                            op[:],
                            lhsT=atbc[ft][qc][:, P * stj:P * (stj + 1)],
                            rhs=wotb[ft][:, QC * oc:QC * (oc + 1)],
                            start=(ft == 0), stop=(ft == 3),
                        )
                    ost = p3.tile([P, QC], F32, tag="ost")
                    nc.vector.tensor_copy(ost[:], op[:])
                    nc.sync.dma_start(y_t[st][:, QC * oc:QC * (oc + 1)], ost[:])

            from collections import deque
            oproj_q = deque()
            pending_norms = []
            for qc in range(NSC):
                nkt = 4 * qc + 4
                for hp in range(4):
                    avpA = ps2.tile([65, QC], F32, tag="avp", bufs=4, name="avpA")
                    avpB = ps2.tile([65, QC], F32, tag="avp", bufs=4, name="avpB")
                    for ki in range(nkt):
                        last = ki == nkt - 1
                        attn_step(hp, qc, ki, avpA, avpB, last)
                    # flush deferred work from the previous group to fill the
                    # PE pipeline while this group's exps/AVs drain
                    prev, pending_norms = pending_norms, [
                        (hp, 0, qc, avpA), (hp, 64, qc, avpB)]
                    for args in prev:
                        normalize(*args)
                    if oproj_q:
                        oproj_piece(*oproj_q.popleft())
                for st in range(4 * qc, 4 * qc + 4):
                    oproj_q.append((qc, st))
            for args in pending_norms:
                normalize(*args)
            while oproj_q:
                oproj_piece(*oproj_q.popleft())

    nc.compile()
    return nc


def _host_tables():
    inv_freq = 1.0 / (10000.0 ** (np.arange(0, HD, 2, dtype=np.float32) / HD))
    pos = np.arange(S, dtype=np.float32)
    freqs = np.einsum('s,d->sd', pos, inv_freq)          # [S, 32]
    emb = np.concatenate([freqs, freqs], axis=-1)        # [S, 64]
    cosT = np.cos(emb).T.astype(np.float32)              # [64, S]
    sinT = np.sin(emb).T.astype(np.float32)
    c2 = np.ascontiguousarray(np.vstack([cosT, cosT]))   # [128, S]
    # sign of rotate_half is encoded in the rot matrix below; ss is plain sin
    ss = np.ascontiguousarray(np.vstack([sinT, sinT]))
    # rotate-half as a matmul: out[d] = sum_d' R[d', d] * in[d']
    R64 = np.zeros((HD, HD), dtype=np.float32)
    for d in range(32):
        R64[d + 32, d] = -1.0       # out[d] = -in[d+32]
        R64[d, d + 32] = 1.0        # out[d+32] = in[d]
    rot = np.zeros((P, P), dtype=np.float32)
    rot[0:64, 0:64] = R64
    rot[64:128, 64:128] = R64
    # causal bias for a diagonal 128x128 tile in scores^T[k, q] layout
    kk = np.arange(P)[:, None]
    qq = np.arange(P)[None, :]
    msk = np.where(kk <= qq, 0.0, MASK_VAL).astype(np.float32)
    import ml_dtypes
    rot = rot.astype(ml_dtypes.bfloat16)   # exact: entries are 0/+-1
    return c2, ss, rot, msk


# q/o head order within a rank block: pair heads (u, u+4) in each 128-row tile
_HEAD_ORDER = [0, 4, 1, 5, 2, 6, 3, 7]


def _make_in_maps(hidden_states, Wq, Wk, Wv, Wo):
    hs = np.ascontiguousarray(np.asarray(hidden_states, dtype=np.float32))
    Wq = np.asarray(Wq, dtype=np.float32)
    Wk = np.asarray(Wk, dtype=np.float32)
    Wv = np.asarray(Wv, dtype=np.float32)
    Wo = np.asarray(Wo, dtype=np.float32)
    c2, ss, rot, msk = _host_tables()
    in_maps = []
    for c in range(8):
        b, r = c // 4, c % 4
        # row indices of Wq (= cols of Wo) for this rank, in device head order
        qrows = np.concatenate([
            np.arange(HD) + (NHL * r + u) * HD for u in _HEAD_ORDER
        ])
        in_maps.append({
            "xt": np.ascontiguousarray(hs[b].T),
            "wqt": np.ascontiguousarray(Wq[qrows, :].T),
            "wkt": np.ascontiguousarray(Wk[NKO * r:NKO * (r + 1), :].T),
            "wvt": np.ascontiguousarray(Wv[NKO * r:NKO * (r + 1), :].T),
            "wot": np.ascontiguousarray(Wo[:, qrows].T),
            "c2": c2, "ss": ss, "msk": msk, "rot": rot,
        })
    return in_maps


_NC = None


def _get_nc():
    global _NC
    if _NC is None:
        _NC = build_nc()
    return _NC


def run_cores(hidden_states, Wq, Wk, Wv, Wo, **run_kwargs):
    """Run the SPMD kernel; returns (out [B,S,H] fp32, BassKernelResults)."""
    nc = _get_nc()
    in_maps = _make_in_maps(hidden_states, Wq, Wk, Wv, Wo)
    res = run_bass_kernel_spmd(nc, in_maps, list(range(8)), **run_kwargs)
    out = np.zeros((B, S, H), dtype=np.float32)
    for c in range(8):
        out[c // 4] += res.results[c]["y"]
    return out, res


def kernel(hidden_states, Wq, Wk, Wv, Wo):
    out, _ = run_cores(hidden_states, Wq, Wk, Wv, Wo)
    return out
